# revision 54
# baseline (speedup 1.0000x reference)
"""Trainium2 Bass kernel for nn_AttnResLayer (sparse_attention).

Computes, for V [N=12, B=4, T=2048, D=1024] fp32:
  K = rmsnorm(V) * norm_weight
  logits[n,b,t] = dot(w_l, K[n,b,t,:])
  alpha = softmax(logits, axis=n)
  out[b,t,d] = sum_n alpha[n,b,t] * V[n,b,t,d]

Sharding: T split across 8 cores (256 tokens/core per b); w_l/norm_weight
replicated (folded into one weight vector host-side). No collectives.

Per-core kernel (per 128-token chunk, natural layout [128 tok, 1024 d]):
  - sum_d V^2 via ScalarE Square activation with fused accum_out
  - sum_d w*V via VectorE scalar_tensor_tensor (fused mult+reduce)
  - rms = exp(-0.5*ln(ss/D + eps)) on ScalarE (one table set with softmax exp)
  - softmax over n (free dim, 12) with fused exp+sum
  - out = sum_n diag(alpha_n) @ V_n on TensorE (float32r fast path),
    accumulated in PSUM

Schedule: the DMA engines are the bottleneck (151.5 us of traffic at
358 GB/s/core vs ~14 us of pipeline head+tail in the naive order), so ALL
output stores are deferred until after the last V load in SP program order.
The ~11.5 us of store traffic then overlaps the last chunk's compute tail
and the kernel ends right at the memory roofline. PE p-state is kept warm
with pacing matmuls (never read) so the final alpha-weighted MAC burst runs
at full clock; the last chunk's output drains and stores in three pieces
(each on its own PSUM tile to avoid WAR serialization) so the tail past the
last MAC is just one small drain + store launch. The replicated w row is
broadcast on-chip via a ones-column matmul and the identity is generated by
the idle Pool engine, shaving the last ~1.6 us of non-essential DMA traffic.
"""

import numpy as np
from contextlib import ExitStack

import concourse.bass as bass
import concourse.bacc as bacc
import concourse.tile as tile
from concourse import mybir
from concourse.bass_utils import run_bass_kernel_spmd

# Pin all activations to the one table set containing exp+ln+square so the
# compiler emits a single ACT_TABLE_LOAD instead of thrashing sets per chunk.
def _pinned_tables(arch, _orig=bacc.get_activation_tables):
    tables = _orig(arch)
    keep = "natural_log_exp_and_others"
    return {k: (v if k == keep else set()) for k, v in tables.items()}

N, B, T, D = 12, 4, 2048, 1024
NCORES = 8
TSH = T // NCORES  # tokens per core (per b)
P = 128            # tokens per chunk (partition dim)
NCHUNK = TSH // P
NCK = B * NCHUNK   # total chunks per core
EPS = 1e-6
FP32 = mybir.dt.float32
FP32R = mybir.dt.float32r
AF = mybir.ActivationFunctionType
ALU = mybir.AluOpType
H = D // 512       # matmul moving-operand free-dim limit is 512 fp32


def _build_nc() -> bacc.Bacc:
    nc = bacc.Bacc("TRN2", target_bir_lowering=False, debug=False,
                   num_devices=NCORES)
    v_in = nc.dram_tensor("v", [N, B, TSH, D], FP32R, kind="ExternalInput").ap()
    wb_in = nc.dram_tensor("wb", [1, D], FP32R, kind="ExternalInput").ap()
    ones_in = nc.dram_tensor("ones", [1, P], FP32R, kind="ExternalInput").ap()
    out_d = nc.dram_tensor("out", [B, TSH, D], FP32, kind="ExternalOutput").ap()

    orig_tables = bacc.get_activation_tables
    bacc.get_activation_tables = _pinned_tables
    try:
        _build_body(nc, v_in, wb_in, ones_in, out_d)
    finally:
        bacc.get_activation_tables = orig_tables
    return nc


def _build_body(nc, v_in, wb_in, ones_in, out_d):
    with tile.TileContext(nc) as tc, ExitStack() as ctx:
        const_pool = ctx.enter_context(tc.tile_pool(name="const", bufs=1))
        v_pool = ctx.enter_context(tc.tile_pool(name="vp", bufs=2))
        scr_pool = ctx.enter_context(tc.tile_pool(name="scr", bufs=1))
        small_pool = ctx.enter_context(tc.tile_pool(name="small", bufs=4))
        diag_pool = ctx.enter_context(tc.tile_pool(name="diag", bufs=16))
        psum_pool = ctx.enter_context(
            tc.tile_pool(name="accp", bufs=2, space="PSUM"))
        warm_pool = ctx.enter_context(
            tc.tile_pool(name="warmp", bufs=1, space="PSUM"))
        out_pool = ctx.enter_context(tc.tile_pool(name="outp", bufs=8))

        eps_t = const_pool.tile([P, 1], FP32, name="eps_t")
        nc.vector.memset(eps_t[:], EPS)
        # w_l*norm_weight comes in as a single [1, D] row; broadcast it across
        # the 128 partitions with a ones-column matmul instead of DMAing a
        # pre-broadcast [128, D] block (saves ~1.4us of HBM-bound DMA time).
        wbsm = const_pool.tile([1, D], FP32R, name="wbsm")
        nc.scalar.dma_start(wbsm[:], wb_in[:])
        ones_t = const_pool.tile([1, P], FP32R, name="ones_t")
        nc.scalar.dma_start(ones_t[:], ones_in[:])
        # [P, P] identity tiles, generated on-chip by the otherwise-idle Pool
        # engine instead of DMAing np.eye from DRAM (the generating
        # instructions are issued inside chunk 0, after its loads; the
        # identity isn't needed until chunk 0's softmax, ~7us in)
        id_one = const_pool.tile([P, P], FP32, name="id_one")
        id_t = const_pool.tile([P, P], FP32R, name="id_t")
        id_r = id_t[:]
        id_f = id_t[:].bitcast(FP32)
        wb_t = const_pool.tile([P, D], FP32, name="wb_t")
        for h in range(H):
            wbp = psum_pool.tile([P, 512], FP32, name="wbp", tag="accl")
            nc.tensor.matmul(wbp[:], ones_t[:], wbsm[:, h * 512:(h + 1) * 512],
                             start=True, stop=True)
            nc.vector.tensor_copy(wb_t[:, h * 512:(h + 1) * 512], wbp[:])
        scr_act = scr_pool.tile([P, D], FP32, name="scr_act")
        scr_dve = scr_pool.tile([P, D], FP32, name="scr_dve")

        stores = []  # (dram dst AP, on-chip src AP), issued after all loads
        for ci in range(NCK):
            b, c = divmod(ci, NCHUNK)
            t0 = c * P
            last = ci == NCK - 1
            # One tile PER SLICE (not one [P, N, D] block): dependency
            # tracking is tile-granular, so per-slice tiles let chunk i+2's
            # load of slice q start as soon as chunk i's readers of slice q
            # are done (its dot/square/warm and the n=q matmuls) instead of
            # waiting for the whole MAC burst — decoupling the load pipeline
            # from the burst latency.
            vslices = []
            for q in range(N):
                vt = v_pool.tile([P, D], FP32R, name=f"vs{q}", tag=f"vs{q}")
                nc.sync.dma_start(vt[:], v_in[q, b, t0:t0 + P, :])
                vslices.append(vt)
            if ci == 0:
                # memset ones, keep only where column index == partition index
                nc.gpsimd.memset(id_one[:], 1.0)
                nc.gpsimd.affine_select(out=id_t[:], in_=id_one[:],
                                        pattern=[[1, P]],
                                        compare_op=ALU.is_equal, fill=0.0,
                                        base=0, channel_multiplier=-1)
            vts = [vslices[n][:].bitcast(FP32) for n in range(N)]
            vts_r = [vslices[n][:] for n in range(N)]

            # PE clock pacing: one throwaway matmul per arriving slice
            # (never read) keeps the HAM/pstate warm between MAC bursts
            warm_ps = warm_pool.tile([P, 512], FP32, name="warm_ps", tag="wp")
            if ci > 0:
                for q in range(N):
                    nc.tensor.matmul(warm_ps[:], id_r,
                                     vts_r[q][:, 0:512],
                                     start=True, stop=True)

            ss = small_pool.tile([P, N], FP32, name="ss", tag="ss")
            dot = small_pool.tile([P, N], FP32, name="dot", tag="dot")
            # gpsimd can't run TensorScalarPtr: squares on ACT, dots on DVE
            for n in range(N):
                nc.scalar.activation(scr_act[:], vts[n], AF.Square,
                                     accum_out=ss[:, n:n + 1])
            for n in range(N):
                nc.vector.scalar_tensor_tensor(
                    out=scr_dve[:], in0=vts[n], scalar=0.0,
                    in1=wb_t[:], op0=ALU.bypass, op1=ALU.mult,
                    accum_out=dot[:, n:n + 1])

            if last:
                # bridge the reduction->softmax window with pacing matmuls so
                # the PE p-state stays at full clock into the final burst
                for _ in range(13):
                    nc.tensor.matmul(warm_ps[:], id_r,
                                     vts_r[N - 1][:, 0:512],
                                     start=True, stop=True)

            # rms = (mean(V^2) + eps)^-0.5 = exp(-0.5*ln(ss/D + eps))
            u = small_pool.tile([P, N], FP32, name="u", tag="u")
            nc.scalar.activation(u[:], ss[:], AF.Ln, bias=eps_t[:, 0:1],
                                 scale=1.0 / D)
            rms = small_pool.tile([P, N], FP32, name="rms", tag="rms")
            nc.scalar.activation(rms[:], u[:], AF.Exp, scale=-0.5)
            logits = small_pool.tile([P, N], FP32, name="logits", tag="lg")
            nc.vector.tensor_mul(logits[:], dot[:], rms[:])

            # softmax over n (free dim): exp(x - max) fused with sum
            negmax = small_pool.tile([P, 1], FP32, name="negmax", tag="nm")
            nc.vector.tensor_reduce(negmax[:], logits[:],
                                    axis=mybir.AxisListType.X,
                                    op=ALU.max, negate=True)
            aexp = small_pool.tile([P, N], FP32, name="aexp", tag="ax")
            sumexp = small_pool.tile([P, 1], FP32, name="sumexp", tag="se")
            nc.scalar.activation(aexp[:], logits[:], AF.Exp,
                                 bias=negmax[:, 0:1], accum_out=sumexp[:])
            recip = small_pool.tile([P, 1], FP32, name="recip", tag="rc")
            nc.vector.reciprocal(recip[:], sumexp[:])

            # diag(alpha_n) tiles; normalization folded in
            dgs = []
            for n in range(N):
                dg = diag_pool.tile([P, P], FP32R, name="dg", tag="dg")
                nc.vector.tensor_scalar(out=dg[:], in0=id_f,
                                        scalar1=aexp[:, n:n + 1],
                                        scalar2=recip[:, 0:1],
                                        op0=ALU.mult, op1=ALU.mult)
                dgs.append(dg)

            # out[t, d] = sum_n alpha[n, t] * V_n[t, d] on TensorE
            out_sb = out_pool.tile([P, D], FP32, name="out_sb", tag="ot")
            if not last:
                acc = psum_pool.tile([P, D], FP32, name="acc", tag="acc")
                for h in range(H):
                    for n in range(N):
                        nc.tensor.matmul(acc[:, h * 512:(h + 1) * 512],
                                         dgs[n][:],
                                         vts_r[n][:, h * 512:(h + 1) * 512],
                                         start=(n == 0), stop=(n == N - 1))
                # Demote the PSUM drain so it doesn't block the next
                # chunk's work in the in-order DVE queue (stores are at the
                # end of the kernel). DVE-drain + 2 V buffers measured as the
                # most robust choice under cost-model perturbation: the
                # slot back-pressure spreads any compute lag smoothly instead
                # of concentrating it where the deferred stores serialize.
                with tc.high_priority(offset=-100):
                    nc.vector.tensor_copy(out_sb[:], acc[:])
                stores.append((out_d[b, t0:t0 + P, :], out_sb[:]))
            else:
                # final chunk: drain each piece while the next piece's MAC
                # chain still runs, shortening the kernel tail. Each piece
                # gets its own PSUM tile so the drain of piece k doesn't
                # create a WAR dependency stalling piece k+1's matmuls.
                for a0, a1 in ((0, 512), (512, 768), (768, 1024)):
                    w = a1 - a0
                    accl = psum_pool.tile([P, 512], FP32, name="accl",
                                          tag="accl")
                    for n in range(N):
                        nc.tensor.matmul(accl[:, 0:w], dgs[n][:],
                                         vts_r[n][:, a0:a1],
                                         start=(n == 0), stop=(n == N - 1))
                    nc.vector.tensor_copy(out_sb[:, a0:a1], accl[:, 0:w])
                    stores.append((out_d[b, t0:t0 + P, a0:a1],
                                   out_sb[:, a0:a1]))

        # All stores issue after the last load on the in-order SP queue, so
        # the ~10us of store traffic covers the final chunk's compute tail.
        for dst, src in stores:
            nc.sync.dma_start(dst, src)
    nc.compile()
    return nc


_NC = None


def _get_nc() -> bacc.Bacc:
    global _NC
    if _NC is None:
        _NC = _build_nc()
    return _NC


def _make_in_maps(V, w_l, norm_weight):
    V = np.ascontiguousarray(np.asarray(V, dtype=np.float32))
    w = np.asarray(w_l, np.float32) * np.asarray(norm_weight, np.float32)
    wb = np.ascontiguousarray(w.reshape(1, D))
    ones = np.ones((1, P), dtype=np.float32)
    in_maps = []
    for c in range(NCORES):
        vs = np.ascontiguousarray(V[:, :, c * TSH:(c + 1) * TSH, :])
        in_maps.append({"v": vs, "wb": wb, "ones": ones})
    return in_maps


def _run(in_maps, trace=False, **kwargs):
    return run_bass_kernel_spmd(_get_nc(), in_maps, list(range(NCORES)),
                                trace=trace, **kwargs)


def kernel(V, w_l, norm_weight):
    res = _run(_make_in_maps(V, w_l, norm_weight))
    outs = [res.results[i]["out"] for i in range(NCORES)]
    return np.concatenate(outs, axis=1).astype(np.float32)


# revision 65
# speedup vs baseline: 1.0013x; 1.0013x over previous
"""Trainium2 Bass kernel for nn_AttnResLayer (sparse_attention).

Computes, for V [N=12, B=4, T=2048, D=1024] fp32:
  K = rmsnorm(V) * norm_weight
  logits[n,b,t] = dot(w_l, K[n,b,t,:])
  alpha = softmax(logits, axis=n)
  out[b,t,d] = sum_n alpha[n,b,t] * V[n,b,t,d]

Sharding: T split across 8 cores (256 tokens/core per b); w_l/norm_weight
replicated (folded into one weight vector host-side). No collectives.

Per-core kernel (per 128-token chunk, natural layout [128 tok, 1024 d]):
  - sum_d V^2 via ScalarE Square activation with fused accum_out
  - sum_d w*V via VectorE scalar_tensor_tensor (fused mult+reduce)
  - rms = exp(-0.5*ln(ss/D + eps)) on ScalarE (one table set with softmax exp)
  - softmax over n (free dim, 12) with fused exp+sum
  - out = sum_n diag(alpha_n) @ V_n on TensorE (float32r fast path),
    accumulated in PSUM

Schedule: the DMA engines are the bottleneck (151.5 us of traffic at
358 GB/s/core vs ~14 us of pipeline head+tail in the naive order), so ALL
output stores are deferred until after the last V load in SP program order.
The ~11.5 us of store traffic then overlaps the last chunk's compute tail
and the kernel ends right at the memory roofline. PE p-state is kept warm
with pacing matmuls (never read) so the final alpha-weighted MAC burst runs
at full clock; the last chunk's output drains and stores in three pieces
(each on its own PSUM tile to avoid WAR serialization) so the tail past the
last MAC is just one small drain + store launch. The replicated w row is
broadcast on-chip via a ones-column matmul and the identity is generated by
the idle Pool engine, shaving the last ~1.6 us of non-essential DMA traffic.
Four 4-byte no-op SP loads phase-align the rotating DMA-completion
semaphores so the teardown's single pending sem-wait is checked last and
the other waits' decodes hide inside it.
"""

import numpy as np
from contextlib import ExitStack

import concourse.bass as bass
import concourse.bacc as bacc
import concourse.tile as tile
from concourse import mybir
from concourse.bass_utils import run_bass_kernel_spmd

# Pin all activations to the one table set containing exp+ln+square so the
# compiler emits a single ACT_TABLE_LOAD instead of thrashing sets per chunk.
def _pinned_tables(arch, _orig=bacc.get_activation_tables):
    tables = _orig(arch)
    keep = "natural_log_exp_and_others"
    return {k: (v if k == keep else set()) for k, v in tables.items()}

N, B, T, D = 12, 4, 2048, 1024
NCORES = 8
TSH = T // NCORES  # tokens per core (per b)
P = 128            # tokens per chunk (partition dim)
NCHUNK = TSH // P
NCK = B * NCHUNK   # total chunks per core
EPS = 1e-6
FP32 = mybir.dt.float32
FP32R = mybir.dt.float32r
AF = mybir.ActivationFunctionType
ALU = mybir.AluOpType
H = D // 512       # matmul moving-operand free-dim limit is 512 fp32


def _build_nc() -> bacc.Bacc:
    nc = bacc.Bacc("TRN2", target_bir_lowering=False, debug=False,
                   num_devices=NCORES)
    v_in = nc.dram_tensor("v", [N, B, TSH, D], FP32R, kind="ExternalInput").ap()
    wb_in = nc.dram_tensor("wb", [1, D], FP32R, kind="ExternalInput").ap()
    ones_in = nc.dram_tensor("ones", [1, P], FP32R, kind="ExternalInput").ap()
    out_d = nc.dram_tensor("out", [B, TSH, D], FP32, kind="ExternalOutput").ap()

    orig_tables = bacc.get_activation_tables
    bacc.get_activation_tables = _pinned_tables
    try:
        _build_body(nc, v_in, wb_in, ones_in, out_d)
    finally:
        bacc.get_activation_tables = orig_tables
    return nc


def _build_body(nc, v_in, wb_in, ones_in, out_d):
    with tile.TileContext(nc) as tc, ExitStack() as ctx:
        const_pool = ctx.enter_context(tc.tile_pool(name="const", bufs=1))
        v_pool = ctx.enter_context(tc.tile_pool(name="vp", bufs=2))
        scr_pool = ctx.enter_context(tc.tile_pool(name="scr", bufs=1))
        small_pool = ctx.enter_context(tc.tile_pool(name="small", bufs=4))
        diag_pool = ctx.enter_context(tc.tile_pool(name="diag", bufs=16))
        psum_pool = ctx.enter_context(
            tc.tile_pool(name="accp", bufs=2, space="PSUM"))
        warm_pool = ctx.enter_context(
            tc.tile_pool(name="warmp", bufs=1, space="PSUM"))
        out_pool = ctx.enter_context(tc.tile_pool(name="outp", bufs=8))

        eps_t = const_pool.tile([P, 1], FP32, name="eps_t")
        nc.vector.memset(eps_t[:], EPS)
        # w_l*norm_weight comes in as a single [1, D] row; broadcast it across
        # the 128 partitions with a ones-column matmul instead of DMAing a
        # pre-broadcast [128, D] block (saves ~1.4us of HBM-bound DMA time).
        wbsm = const_pool.tile([1, D], FP32R, name="wbsm")
        nc.scalar.dma_start(wbsm[:], wb_in[:])
        ones_t = const_pool.tile([1, P], FP32R, name="ones_t")
        nc.scalar.dma_start(ones_t[:], ones_in[:])
        # [P, P] identity tiles, generated on-chip by the otherwise-idle Pool
        # engine instead of DMAing np.eye from DRAM (the generating
        # instructions are issued inside chunk 0, after its loads; the
        # identity isn't needed until chunk 0's softmax, ~7us in)
        id_one = const_pool.tile([P, P], FP32, name="id_one")
        id_t = const_pool.tile([P, P], FP32R, name="id_t")
        id_r = id_t[:]
        id_f = id_t[:].bitcast(FP32)
        wb_t = const_pool.tile([P, D], FP32, name="wb_t")
        for h in range(H):
            wbp = psum_pool.tile([P, 512], FP32, name="wbp", tag="accl")
            nc.tensor.matmul(wbp[:], ones_t[:], wbsm[:, h * 512:(h + 1) * 512],
                             start=True, stop=True)
            nc.vector.tensor_copy(wb_t[:, h * 512:(h + 1) * 512], wbp[:])
        scr_act = scr_pool.tile([P, D], FP32, name="scr_act")
        scr_dve = scr_pool.tile([P, D], FP32, name="scr_dve")

        stores = []  # (dram dst AP, on-chip src AP), issued after all loads
        for ci in range(NCK):
            b, c = divmod(ci, NCHUNK)
            t0 = c * P
            last = ci == NCK - 1
            # One tile PER SLICE (not one [P, N, D] block): dependency
            # tracking is tile-granular, so per-slice tiles let chunk i+2's
            # load of slice q start as soon as chunk i's readers of slice q
            # are done (its dot/square/warm and the n=q matmuls) instead of
            # waiting for the whole MAC burst — decoupling the load pipeline
            # from the burst latency.
            vslices = []
            for q in range(N):
                vt = v_pool.tile([P, D], FP32R, name=f"vs{q}", tag=f"vs{q}")
                nc.sync.dma_start(vt[:], v_in[q, b, t0:t0 + P, :])
                vslices.append(vt)
            if ci == 0:
                # memset ones, keep only where column index == partition index
                nc.gpsimd.memset(id_one[:], 1.0)
                nc.gpsimd.affine_select(out=id_t[:], in_=id_one[:],
                                        pattern=[[1, P]],
                                        compare_op=ALU.is_equal, fill=0.0,
                                        base=0, channel_multiplier=-1)
            if ci == 2:
                dmy = const_pool.tile([1, 1], FP32R, name="dmy")
            if 2 <= ci <= 5:
                # phase-shift SP's rotating DMA-completion semaphores with 4
                # no-op loads (~7ns of bus time each, one per chunk so their
                # WAW chain never blocks the SP queue) so the FINAL store
                # lands on the last-checked sem: the teardown's other
                # sem-wait decodes then hide inside the one pending wait
                nc.sync.dma_start(dmy[:], wb_in[0:1, 0:1])
            vts = [vslices[n][:].bitcast(FP32) for n in range(N)]
            vts_r = [vslices[n][:] for n in range(N)]

            # PE clock pacing: one throwaway matmul per arriving slice
            # (never read) keeps the HAM/pstate warm between MAC bursts
            warm_ps = warm_pool.tile([P, 512], FP32, name="warm_ps", tag="wp")
            if ci > 0:
                for q in range(N):
                    nc.tensor.matmul(warm_ps[:], id_r,
                                     vts_r[q][:, 0:512],
                                     start=True, stop=True)

            ss = small_pool.tile([P, N], FP32, name="ss", tag="ss")
            dot = small_pool.tile([P, N], FP32, name="dot", tag="dot")
            # gpsimd can't run TensorScalarPtr: squares on ACT, dots on DVE
            for n in range(N):
                nc.scalar.activation(scr_act[:], vts[n], AF.Square,
                                     accum_out=ss[:, n:n + 1])
            for n in range(N):
                nc.vector.scalar_tensor_tensor(
                    out=scr_dve[:], in0=vts[n], scalar=0.0,
                    in1=wb_t[:], op0=ALU.bypass, op1=ALU.mult,
                    accum_out=dot[:, n:n + 1])

            if last:
                # bridge the reduction->softmax window with pacing matmuls so
                # the PE p-state stays at full clock into the final burst
                for _ in range(13):
                    nc.tensor.matmul(warm_ps[:], id_r,
                                     vts_r[N - 1][:, 0:512],
                                     start=True, stop=True)

            # rms = (mean(V^2) + eps)^-0.5 = exp(-0.5*ln(ss/D + eps))
            u = small_pool.tile([P, N], FP32, name="u", tag="u")
            nc.scalar.activation(u[:], ss[:], AF.Ln, bias=eps_t[:, 0:1],
                                 scale=1.0 / D)
            rms = small_pool.tile([P, N], FP32, name="rms", tag="rms")
            nc.scalar.activation(rms[:], u[:], AF.Exp, scale=-0.5)
            logits = small_pool.tile([P, N], FP32, name="logits", tag="lg")
            nc.vector.tensor_mul(logits[:], dot[:], rms[:])

            # softmax over n (free dim): exp(x - max) fused with sum
            negmax = small_pool.tile([P, 1], FP32, name="negmax", tag="nm")
            nc.vector.tensor_reduce(negmax[:], logits[:],
                                    axis=mybir.AxisListType.X,
                                    op=ALU.max, negate=True)
            aexp = small_pool.tile([P, N], FP32, name="aexp", tag="ax")
            sumexp = small_pool.tile([P, 1], FP32, name="sumexp", tag="se")
            nc.scalar.activation(aexp[:], logits[:], AF.Exp,
                                 bias=negmax[:, 0:1], accum_out=sumexp[:])
            recip = small_pool.tile([P, 1], FP32, name="recip", tag="rc")
            nc.vector.reciprocal(recip[:], sumexp[:])

            # diag(alpha_n) tiles; normalization folded in
            dgs = []
            for n in range(N):
                dg = diag_pool.tile([P, P], FP32R, name="dg", tag="dg")
                nc.vector.tensor_scalar(out=dg[:], in0=id_f,
                                        scalar1=aexp[:, n:n + 1],
                                        scalar2=recip[:, 0:1],
                                        op0=ALU.mult, op1=ALU.mult)
                dgs.append(dg)

            # out[t, d] = sum_n alpha[n, t] * V_n[t, d] on TensorE
            out_sb = out_pool.tile([P, D], FP32, name="out_sb", tag="ot")
            if not last:
                acc = psum_pool.tile([P, D], FP32, name="acc", tag="acc")
                for h in range(H):
                    for n in range(N):
                        nc.tensor.matmul(acc[:, h * 512:(h + 1) * 512],
                                         dgs[n][:],
                                         vts_r[n][:, h * 512:(h + 1) * 512],
                                         start=(n == 0), stop=(n == N - 1))
                # Demote the PSUM drain so it doesn't block the next
                # chunk's work in the in-order DVE queue (stores are at the
                # end of the kernel). DVE-drain + 2 V buffers measured as the
                # most robust choice under cost-model perturbation: the
                # slot back-pressure spreads any compute lag smoothly instead
                # of concentrating it where the deferred stores serialize.
                with tc.high_priority(offset=-100):
                    nc.vector.tensor_copy(out_sb[:], acc[:])
                stores.append((out_d[b, t0:t0 + P, :], out_sb[:]))
            else:
                # final chunk: drain each piece while the next piece's MAC
                # chain still runs, shortening the kernel tail. Each piece
                # gets its own PSUM tile so the drain of piece k doesn't
                # create a WAR dependency stalling piece k+1's matmuls.
                for a0, a1 in ((0, 512), (512, 768), (768, 1024)):
                    w = a1 - a0
                    accl = psum_pool.tile([P, 512], FP32, name="accl",
                                          tag="accl")
                    for n in range(N):
                        nc.tensor.matmul(accl[:, 0:w], dgs[n][:],
                                         vts_r[n][:, a0:a1],
                                         start=(n == 0), stop=(n == N - 1))
                    nc.vector.tensor_copy(out_sb[:, a0:a1], accl[:, 0:w])
                    stores.append((out_d[b, t0:t0 + P, a0:a1],
                                   out_sb[:, a0:a1]))

        # All stores issue after the last load on the in-order SP queue, so
        # the ~10us of store traffic covers the final chunk's compute tail.
        for dst, src in stores:
            nc.sync.dma_start(dst, src)
    nc.compile()
    return nc


_NC = None


def _get_nc() -> bacc.Bacc:
    global _NC
    if _NC is None:
        _NC = _build_nc()
    return _NC


def _make_in_maps(V, w_l, norm_weight):
    V = np.ascontiguousarray(np.asarray(V, dtype=np.float32))
    w = np.asarray(w_l, np.float32) * np.asarray(norm_weight, np.float32)
    wb = np.ascontiguousarray(w.reshape(1, D))
    ones = np.ones((1, P), dtype=np.float32)
    in_maps = []
    for c in range(NCORES):
        vs = np.ascontiguousarray(V[:, :, c * TSH:(c + 1) * TSH, :])
        in_maps.append({"v": vs, "wb": wb, "ones": ones})
    return in_maps


def _run(in_maps, trace=False, **kwargs):
    return run_bass_kernel_spmd(_get_nc(), in_maps, list(range(NCORES)),
                                trace=trace, **kwargs)


def kernel(V, w_l, norm_weight):
    res = _run(_make_in_maps(V, w_l, norm_weight))
    outs = [res.results[i]["out"] for i in range(NCORES)]
    return np.concatenate(outs, axis=1).astype(np.float32)


# revision 69
# speedup vs baseline: 1.0014x; 1.0001x over previous
"""Trainium2 Bass kernel for nn_AttnResLayer (sparse_attention).

Computes, for V [N=12, B=4, T=2048, D=1024] fp32:
  K = rmsnorm(V) * norm_weight
  logits[n,b,t] = dot(w_l, K[n,b,t,:])
  alpha = softmax(logits, axis=n)
  out[b,t,d] = sum_n alpha[n,b,t] * V[n,b,t,d]

Sharding: T split across 8 cores (256 tokens/core per b); w_l/norm_weight
replicated (folded into one weight vector host-side). No collectives.

Per-core kernel (per 128-token chunk, natural layout [128 tok, 1024 d]):
  - sum_d V^2 via ScalarE Square activation with fused accum_out
  - sum_d w*V via VectorE scalar_tensor_tensor (fused mult+reduce)
  - rms = exp(-0.5*ln(ss/D + eps)) on ScalarE (one table set with softmax exp)
  - softmax over n (free dim, 12) with fused exp+sum
  - out = sum_n diag(alpha_n) @ V_n on TensorE (float32r fast path),
    accumulated in PSUM

Schedule: the DMA engines are the bottleneck (151.5 us of traffic at
358 GB/s/core vs ~14 us of pipeline head+tail in the naive order), so ALL
output stores are deferred until after the last V load in SP program order.
The ~11.5 us of store traffic then overlaps the last chunk's compute tail
and the kernel ends right at the memory roofline. PE p-state is kept warm
with pacing matmuls (never read) so the final alpha-weighted MAC burst runs
at full clock; the last chunk's output drains and stores in three pieces
(each on its own PSUM tile to avoid WAR serialization) so the tail past the
last MAC is just one small drain + store launch. The replicated w row is
broadcast on-chip via a ones-column matmul and the identity is generated by
the idle Pool engine, shaving the last ~1.6 us of non-essential DMA traffic.
Four 4-byte no-op SP loads phase-align the rotating DMA-completion
semaphores so the teardown's single pending sem-wait is checked last and
the other waits' decodes hide inside it.
"""

import numpy as np
from contextlib import ExitStack

import concourse.bass as bass
import concourse.bacc as bacc
import concourse.tile as tile
from concourse import mybir
from concourse.bass_utils import run_bass_kernel_spmd

# Pin all activations to the one table set containing exp+ln+square so the
# compiler emits a single ACT_TABLE_LOAD instead of thrashing sets per chunk.
def _pinned_tables(arch, _orig=bacc.get_activation_tables):
    tables = _orig(arch)
    keep = "natural_log_exp_and_others"
    return {k: (v if k == keep else set()) for k, v in tables.items()}

N, B, T, D = 12, 4, 2048, 1024
NCORES = 8
TSH = T // NCORES  # tokens per core (per b)
P = 128            # tokens per chunk (partition dim)
NCHUNK = TSH // P
NCK = B * NCHUNK   # total chunks per core
EPS = 1e-6
FP32 = mybir.dt.float32
FP32R = mybir.dt.float32r
AF = mybir.ActivationFunctionType
ALU = mybir.AluOpType
H = D // 512       # matmul moving-operand free-dim limit is 512 fp32


def _build_nc() -> bacc.Bacc:
    nc = bacc.Bacc("TRN2", target_bir_lowering=False, debug=False,
                   num_devices=NCORES)
    v_in = nc.dram_tensor("v", [N, B, TSH, D], FP32R, kind="ExternalInput").ap()
    wb_in = nc.dram_tensor("wb", [2, D // 2], FP32R, kind="ExternalInput").ap()
    ones_in = nc.dram_tensor("ones", [2, P], FP32R, kind="ExternalInput").ap()
    out_d = nc.dram_tensor("out", [B, TSH, D], FP32, kind="ExternalOutput").ap()

    orig_tables = bacc.get_activation_tables
    bacc.get_activation_tables = _pinned_tables
    try:
        _build_body(nc, v_in, wb_in, ones_in, out_d)
    finally:
        bacc.get_activation_tables = orig_tables
    return nc


def _build_body(nc, v_in, wb_in, ones_in, out_d):
    with tile.TileContext(nc) as tc, ExitStack() as ctx:
        const_pool = ctx.enter_context(tc.tile_pool(name="const", bufs=1))
        v_pool = ctx.enter_context(tc.tile_pool(name="vp", bufs=2))
        scr_pool = ctx.enter_context(tc.tile_pool(name="scr", bufs=1))
        small_pool = ctx.enter_context(tc.tile_pool(name="small", bufs=4))
        diag_pool = ctx.enter_context(tc.tile_pool(name="diag", bufs=16))
        psum_pool = ctx.enter_context(
            tc.tile_pool(name="accp", bufs=2, space="PSUM"))
        warm_pool = ctx.enter_context(
            tc.tile_pool(name="warmp", bufs=1, space="PSUM"))
        out_pool = ctx.enter_context(tc.tile_pool(name="outp", bufs=8))

        eps_t = const_pool.tile([P, 1], FP32, name="eps_t")
        nc.vector.memset(eps_t[:], EPS)
        # w_l*norm_weight comes in as a single [1, D] row; broadcast it across
        # the 128 partitions with a ones-column matmul instead of DMAing a
        # pre-broadcast [128, D] block (saves ~1.4us of HBM-bound DMA time).
        wbsm = const_pool.tile([33, D // 2], FP32R, name="wbsm")
        nc.scalar.dma_start(wbsm[0:33:32, :], wb_in[:])
        ones_t = const_pool.tile([33, P], FP32R, name="ones_t")
        nc.scalar.dma_start(ones_t[0:33:32, :], ones_in[:])
        # [P, P] identity tiles, generated on-chip by the otherwise-idle Pool
        # engine instead of DMAing np.eye from DRAM (the generating
        # instructions are issued inside chunk 0, after its loads; the
        # identity isn't needed until chunk 0's softmax, ~7us in)
        id_one = const_pool.tile([P, P], FP32, name="id_one")
        id_t = const_pool.tile([P, P], FP32R, name="id_t")
        id_r = id_t[:]
        id_f = id_t[:].bitcast(FP32)
        wb_t = const_pool.tile([P, D], FP32, name="wb_t")
        for h in range(H):
            wbp = psum_pool.tile([P, 512], FP32, name="wbp", tag="accl")
            nc.tensor.matmul(wbp[:], ones_t[h * 32:h * 32 + 1, :],
                             wbsm[h * 32:h * 32 + 1, :],
                             start=True, stop=True)
            nc.vector.tensor_copy(wb_t[:, h * 512:(h + 1) * 512], wbp[:])
        scr_act = scr_pool.tile([P, D], FP32, name="scr_act")
        scr_dve = scr_pool.tile([P, D], FP32, name="scr_dve")

        stores = []  # (dram dst AP, on-chip src AP), issued after all loads
        for ci in range(NCK):
            b, c = divmod(ci, NCHUNK)
            t0 = c * P
            last = ci == NCK - 1
            # One tile PER SLICE (not one [P, N, D] block): dependency
            # tracking is tile-granular, so per-slice tiles let chunk i+2's
            # load of slice q start as soon as chunk i's readers of slice q
            # are done (its dot/square/warm and the n=q matmuls) instead of
            # waiting for the whole MAC burst — decoupling the load pipeline
            # from the burst latency.
            vslices = []
            for q in range(N):
                vt = v_pool.tile([P, D], FP32R, name=f"vs{q}", tag=f"vs{q}")
                nc.sync.dma_start(vt[:], v_in[q, b, t0:t0 + P, :])
                vslices.append(vt)
            if ci == 0:
                # memset ones, keep only where column index == partition index
                nc.gpsimd.memset(id_one[:], 1.0)
                nc.gpsimd.affine_select(out=id_t[:], in_=id_one[:],
                                        pattern=[[1, P]],
                                        compare_op=ALU.is_equal, fill=0.0,
                                        base=0, channel_multiplier=-1)
            if ci == 2:
                dmy = const_pool.tile([1, 1], FP32R, name="dmy")
            if 2 <= ci <= 5:
                # phase-shift SP's rotating DMA-completion semaphores with 4
                # no-op loads (~7ns of bus time each, one per chunk so their
                # WAW chain never blocks the SP queue) so the FINAL store
                # lands on the last-checked sem: the teardown's other
                # sem-wait decodes then hide inside the one pending wait
                nc.sync.dma_start(dmy[:], wb_in[0:1, 0:1])
            vts = [vslices[n][:].bitcast(FP32) for n in range(N)]
            vts_r = [vslices[n][:] for n in range(N)]

            # PE clock pacing: one throwaway matmul per arriving slice
            # (never read) keeps the HAM/pstate warm between MAC bursts
            warm_ps = warm_pool.tile([P, 512], FP32, name="warm_ps", tag="wp")
            if ci > 0:
                for q in range(N):
                    nc.tensor.matmul(warm_ps[:], id_r,
                                     vts_r[q][:, 0:512],
                                     start=True, stop=True)

            ss = small_pool.tile([P, N], FP32, name="ss", tag="ss")
            dot = small_pool.tile([P, N], FP32, name="dot", tag="dot")
            # gpsimd can't run TensorScalarPtr: squares on ACT, dots on DVE
            for n in range(N):
                nc.scalar.activation(scr_act[:], vts[n], AF.Square,
                                     accum_out=ss[:, n:n + 1])
            for n in range(N):
                nc.vector.scalar_tensor_tensor(
                    out=scr_dve[:], in0=vts[n], scalar=0.0,
                    in1=wb_t[:], op0=ALU.bypass, op1=ALU.mult,
                    accum_out=dot[:, n:n + 1])

            if last:
                # bridge the reduction->softmax window with pacing matmuls so
                # the PE p-state stays at full clock into the final burst
                for _ in range(13):
                    nc.tensor.matmul(warm_ps[:], id_r,
                                     vts_r[N - 1][:, 0:512],
                                     start=True, stop=True)

            # rms = (mean(V^2) + eps)^-0.5 = exp(-0.5*ln(ss/D + eps))
            u = small_pool.tile([P, N], FP32, name="u", tag="u")
            nc.scalar.activation(u[:], ss[:], AF.Ln, bias=eps_t[:, 0:1],
                                 scale=1.0 / D)
            rms = small_pool.tile([P, N], FP32, name="rms", tag="rms")
            nc.scalar.activation(rms[:], u[:], AF.Exp, scale=-0.5)
            logits = small_pool.tile([P, N], FP32, name="logits", tag="lg")
            nc.vector.tensor_mul(logits[:], dot[:], rms[:])

            # softmax over n (free dim): exp(x - max) fused with sum
            negmax = small_pool.tile([P, 1], FP32, name="negmax", tag="nm")
            nc.vector.tensor_reduce(negmax[:], logits[:],
                                    axis=mybir.AxisListType.X,
                                    op=ALU.max, negate=True)
            aexp = small_pool.tile([P, N], FP32, name="aexp", tag="ax")
            sumexp = small_pool.tile([P, 1], FP32, name="sumexp", tag="se")
            nc.scalar.activation(aexp[:], logits[:], AF.Exp,
                                 bias=negmax[:, 0:1], accum_out=sumexp[:])
            recip = small_pool.tile([P, 1], FP32, name="recip", tag="rc")
            nc.vector.reciprocal(recip[:], sumexp[:])

            # diag(alpha_n) tiles; normalization folded in
            dgs = []
            for n in range(N):
                dg = diag_pool.tile([P, P], FP32R, name="dg", tag="dg")
                nc.vector.tensor_scalar(out=dg[:], in0=id_f,
                                        scalar1=aexp[:, n:n + 1],
                                        scalar2=recip[:, 0:1],
                                        op0=ALU.mult, op1=ALU.mult)
                dgs.append(dg)

            # out[t, d] = sum_n alpha[n, t] * V_n[t, d] on TensorE
            out_sb = out_pool.tile([P, D], FP32, name="out_sb", tag="ot")
            if not last:
                acc = psum_pool.tile([P, D], FP32, name="acc", tag="acc")
                for h in range(H):
                    for n in range(N):
                        nc.tensor.matmul(acc[:, h * 512:(h + 1) * 512],
                                         dgs[n][:],
                                         vts_r[n][:, h * 512:(h + 1) * 512],
                                         start=(n == 0), stop=(n == N - 1))
                # Demote the PSUM drain so it doesn't block the next
                # chunk's work in the in-order DVE queue (stores are at the
                # end of the kernel). DVE-drain + 2 V buffers measured as the
                # most robust choice under cost-model perturbation: the
                # slot back-pressure spreads any compute lag smoothly instead
                # of concentrating it where the deferred stores serialize.
                with tc.high_priority(offset=-100):
                    nc.vector.tensor_copy(out_sb[:], acc[:])
                stores.append((out_d[b, t0:t0 + P, :], out_sb[:]))
            else:
                # final chunk: drain each piece while the next piece's MAC
                # chain still runs, shortening the kernel tail. Each piece
                # gets its own PSUM tile so the drain of piece k doesn't
                # create a WAR dependency stalling piece k+1's matmuls.
                for a0, a1 in ((0, 512), (512, 768), (768, 1024)):
                    w = a1 - a0
                    accl = psum_pool.tile([P, 512], FP32, name="accl",
                                          tag="accl")
                    for n in range(N):
                        nc.tensor.matmul(accl[:, 0:w], dgs[n][:],
                                         vts_r[n][:, a0:a1],
                                         start=(n == 0), stop=(n == N - 1))
                    nc.vector.tensor_copy(out_sb[:, a0:a1], accl[:, 0:w])
                    stores.append((out_d[b, t0:t0 + P, a0:a1],
                                   out_sb[:, a0:a1]))

        # All stores issue after the last load on the in-order SP queue, so
        # the ~10us of store traffic covers the final chunk's compute tail.
        for dst, src in stores:
            nc.sync.dma_start(dst, src)
    nc.compile()
    return nc


_NC = None


def _get_nc() -> bacc.Bacc:
    global _NC
    if _NC is None:
        _NC = _build_nc()
    return _NC


def _make_in_maps(V, w_l, norm_weight):
    V = np.ascontiguousarray(np.asarray(V, dtype=np.float32))
    w = np.asarray(w_l, np.float32) * np.asarray(norm_weight, np.float32)
    wb = np.ascontiguousarray(w.reshape(2, D // 2))
    ones = np.ones((2, P), dtype=np.float32)
    in_maps = []
    for c in range(NCORES):
        vs = np.ascontiguousarray(V[:, :, c * TSH:(c + 1) * TSH, :])
        in_maps.append({"v": vs, "wb": wb, "ones": ones})
    return in_maps


def _run(in_maps, trace=False, **kwargs):
    return run_bass_kernel_spmd(_get_nc(), in_maps, list(range(NCORES)),
                                trace=trace, **kwargs)


def kernel(V, w_l, norm_weight):
    res = _run(_make_in_maps(V, w_l, norm_weight))
    outs = [res.results[i]["out"] for i in range(NCORES)]
    return np.concatenate(outs, axis=1).astype(np.float32)
